# revision 2
# baseline (speedup 1.0000x reference)
"""Trainium2 Bass kernel for nn_EquiLinearRegToReg.

Math: out[b,p,j,y] = sum_{i,x} weights[i, j, (y-x)%K] * field_feat[b,p,i,x]
Shapes: field_feat [8, 512, 256, 16] f32, weights [256, 256, 16] f32
        -> out [8, 512, 256, 16] f32.

Strategy: data-parallel over batch (1 batch of M=512 rows per core).
Per core this is a [512, 4096] @ [4096, 4096] matmul where the 4096x4096
operand is the block-circulant expansion of weights. We never materialize
that expansion: the moving operand for K-tile (x, io) is a contiguous
16-wide window of WD[io, i, j, s2] = weights[io*128+i, j, s2 % 16]
(s2 in [0,32)) -- window [16-x, 32-x) realizes the circular shift.

Matmuls run in float32r (fp32 data truncated to fp22 in the PE) which
streams at 1 column/cycle like bf16; accumulation is fp32 in PSUM.
"""

import os
import numpy as np

import concourse.bass as bass
import concourse.mybir as mybir
import concourse.tile as tile
from concourse import bacc
from concourse.bass_utils import run_bass_kernel_spmd

BATCH, NUM_PART, IN_FEAT, OUT_FEAT, K = 8, 512, 256, 256, 16
N_CORES = 8
P = 128
IO = IN_FEAT // P          # 2 partition-tiles over in_features
KT = K * IO                # 32 K-tiles of 128
BPC = NUM_PART // P        # 4 chunks of 128 rows
NT = OUT_FEAT * K // 512   # 8 output column tiles of 512
JPN = OUT_FEAT // NT       # 32 j's per output tile

_CACHE = {}


def _build():
    """Build + compile the per-core Bass program (cached)."""
    if "nc" in _CACHE:
        return _CACHE["nc"]

    f32 = mybir.dt.float32
    f32r = mybir.dt.float32r

    nc = bacc.Bacc(None, target_bir_lowering=False, debug=False)
    # fieldT[kt, i128, bp] : K-major transposed field shard, kt = x*IO + io
    field_d = nc.dram_tensor("fieldT", [KT, P, NUM_PART], f32r, kind="ExternalInput")
    # wd[io, nt, i128, j32, s2] : duplicated weights windows
    wd_d = nc.dram_tensor("wd", [IO, NT, P, JPN, 2 * K], f32r, kind="ExternalInput")
    out_d = nc.dram_tensor("out", [NUM_PART, OUT_FEAT * K], f32, kind="ExternalOutput")

    with tile.TileContext(nc) as tc:
        with (
            tc.tile_pool(name="fpool", bufs=1) as fpool,
            tc.tile_pool(name="wpool", bufs=1) as wpool,
            tc.tile_pool(name="opool", bufs=4) as opool,
            tc.tile_pool(name="psum", bufs=8, space="PSUM") as psum,
        ):
            ft = fpool.tile([P, KT, NUM_PART], f32r)
            wdt = wpool.tile([P, IO, NT, JPN, 2 * K], f32r)

            # Interleave the load order so the first (kt=0) matmuls can
            # start after ~1MB of DMA; the rest overlaps with compute.
            nc.sync.dma_start(wdt[:, 0, 0], wd_d[0, 0])
            nc.sync.dma_start(ft[:, 0, :], field_d[0])
            for nt in range(1, NT):
                nc.sync.dma_start(wdt[:, 0, nt], wd_d[0, nt])
            for kt in range(1, KT):
                nc.sync.dma_start(ft[:, kt, :], field_d[kt])
            for nt in range(NT):
                nc.sync.dma_start(wdt[:, 1, nt], wd_d[1, nt])

            for bpc in range(BPC):
                accs = [
                    psum.tile([P, 512], f32, tag="ps", name=f"ps_{bpc}_{i}")
                    for i in range(NT)
                ]
                for kt in range(KT):
                    # kt = x*IO + io: all K-tiles of a given x are adjacent;
                    # io=0 tiles only need the first half of wd.
                    x, io = divmod(kt, IO)
                    lhsT = ft[:, kt, bpc * P:(bpc + 1) * P]
                    for nt in range(NT):
                        rhs = wdt[:, io, nt, :, K - x:2 * K - x]
                        nc.tensor.matmul(
                            accs[nt][:],
                            lhsT,
                            rhs,
                            start=(kt == 0),
                            stop=(kt == KT - 1),
                        )
                for nt in range(NT):
                    ot = opool.tile([P, 512], f32, tag="ot")
                    nc.vector.tensor_copy(ot[:], accs[nt][:])
                    nc.sync.dma_start(
                        out_d[bpc * P:(bpc + 1) * P, nt * 512:(nt + 1) * 512],
                        ot[:],
                    )

    nc.compile()
    _CACHE["nc"] = nc
    return nc


def _prep_inputs(field_feat: np.ndarray, weights: np.ndarray):
    field_feat = np.ascontiguousarray(field_feat, dtype=np.float32)
    weights = np.ascontiguousarray(weights, dtype=np.float32)

    # WD[io, nt, i, j32, s2] = weights[io*128+i, nt*32+j32, s2 % 16]
    wdd = np.concatenate([weights, weights], axis=-1)          # [256, 256, 32]
    wd = wdd.reshape(IO, P, NT, JPN, 2 * K).transpose(0, 2, 1, 3, 4)
    wd = np.ascontiguousarray(wd)

    in_maps = []
    for c in range(N_CORES):
        # fieldT[kt, i128, bp]; kt = x*IO + io over (x, io), i128 minor of i
        fT = field_feat[c].transpose(2, 1, 0)                  # [16x, 256i, 512bp]
        fT = fT.reshape(K, IO, P, NUM_PART).reshape(KT, P, NUM_PART)
        in_maps.append({"fieldT": np.ascontiguousarray(fT), "wd": wd})
    return in_maps


def kernel(field_feat: np.ndarray, weights: np.ndarray) -> np.ndarray:
    nc = _build()
    in_maps = _prep_inputs(field_feat, weights)
    trace = bool(int(os.environ.get("KERNEL_TRACE", "0")))
    res = run_bass_kernel_spmd(nc, in_maps, list(range(N_CORES)), trace=trace)
    if trace:
        kernel.last_exec_time_ns = res.exec_time_ns
        kernel.last_results = res
    out = np.stack([res.results[c]["out"] for c in range(N_CORES)], axis=0)
    return out.reshape(BATCH, NUM_PART, OUT_FEAT, K)


# revision 5
# speedup vs baseline: 1.1396x; 1.1396x over previous
"""Trainium2 Bass kernel for nn_EquiLinearRegToReg.

Math: out[b,p,j,y] = sum_{i,x} weights[i, j, (y-x)%K] * field_feat[b,p,i,x]
Shapes: field_feat [8, 512, 256, 16] f32, weights [256, 256, 16] f32
        -> out [8, 512, 256, 16] f32.

Strategy: data-parallel over batch (1 batch of M=512 rows per core).
Per core this is a [512, 4096] @ [4096, 4096] matmul where the 4096x4096
operand is the block-circulant expansion of weights. We never materialize
that expansion: the moving operand for K-tile (x, io) is a contiguous
16-wide window of WD[io, i, j, s2] = weights[io*128+i, j, s2 % 16]
(s2 in [0,32)) -- window [16-x, 32-x) realizes the circular shift.

Matmuls run in float32r (fp32 data truncated to fp22 in the PE) which
streams at 1 column/cycle like bf16; accumulation is fp32 in PSUM.
"""

import os
import numpy as np

import concourse.bass as bass
import concourse.mybir as mybir
import concourse.tile as tile
from concourse import bacc
from concourse.bass_utils import run_bass_kernel_spmd

BATCH, NUM_PART, IN_FEAT, OUT_FEAT, K = 8, 512, 256, 256, 16
N_CORES = 8
P = 128
IO = IN_FEAT // P          # 2 partition-tiles over in_features
KT = K * IO                # 32 K-tiles of 128
BPC = NUM_PART // P        # 4 chunks of 128 rows
NT = OUT_FEAT * K // 512   # 8 output column tiles of 512
JPN = OUT_FEAT // NT       # 32 j's per output tile

_CACHE = {}

# stationary (field) dtype, moving (wd) dtype. fp16 stationary gets a
# separate FWL LDWEIGHTS that pipelines under the previous matmul; fp32r
# self-loading matmuls serialize a ~107ns weight load per MM.
FIELD_DT = os.environ.get("KERNEL_FIELD_DT", "float16")
WD_DT = os.environ.get("KERNEL_WD_DT", "float32r")

_NP_DT = {"float16": np.float16, "float32r": np.float32, "bfloat16": None}


def _build():
    """Build + compile the per-core Bass program (cached)."""
    if "nc" in _CACHE:
        return _CACHE["nc"]

    f32 = mybir.dt.float32
    fdt = mybir.dt(FIELD_DT)
    wdt_dt = mybir.dt(WD_DT)

    nc = bacc.Bacc(None, target_bir_lowering=False, debug=False)
    # fieldT[kt, i128, bp] : K-major transposed field shard, kt = x*IO + io
    field_d = nc.dram_tensor("fieldT", [KT, P, NUM_PART], fdt, kind="ExternalInput")
    # wd[io, nt, i128, j32, s2] : duplicated weights windows
    wd_d = nc.dram_tensor("wd", [IO, NT, P, JPN, 2 * K], wdt_dt, kind="ExternalInput")
    out_d = nc.dram_tensor("out", [NUM_PART, OUT_FEAT * K], f32, kind="ExternalOutput")

    with tile.TileContext(nc) as tc:
        with (
            tc.tile_pool(name="fpool", bufs=1) as fpool,
            tc.tile_pool(name="wpool", bufs=1) as wpool,
            tc.tile_pool(name="opool", bufs=4) as opool,
            tc.tile_pool(name="psum", bufs=8, space="PSUM") as psum,
        ):
            ft = fpool.tile([P, KT, NUM_PART], fdt)
            wdt = wpool.tile([P, IO, NT, JPN, 2 * K], wdt_dt)

            # Interleave the load order so the first (kt=0) matmuls can
            # start after ~1MB of DMA; the rest overlaps with compute.
            nc.sync.dma_start(wdt[:, 0, 0], wd_d[0, 0])
            nc.sync.dma_start(ft[:, 0, :], field_d[0])
            for nt in range(1, NT):
                nc.sync.dma_start(wdt[:, 0, nt], wd_d[0, nt])
            for kt in range(1, KT):
                nc.sync.dma_start(ft[:, kt, :], field_d[kt])
            for nt in range(NT):
                nc.sync.dma_start(wdt[:, 1, nt], wd_d[1, nt])

            for bpc in range(BPC):
                accs = [
                    psum.tile([P, 512], f32, tag="ps", name=f"ps_{bpc}_{i}")
                    for i in range(NT)
                ]
                for kt in range(KT):
                    # kt = x*IO + io: all K-tiles of a given x are adjacent;
                    # io=0 tiles only need the first half of wd.
                    x, io = divmod(kt, IO)
                    lhsT = ft[:, kt, bpc * P:(bpc + 1) * P]
                    for nt in range(NT):
                        rhs = wdt[:, io, nt, :, K - x:2 * K - x]
                        nc.tensor.matmul(
                            accs[nt][:],
                            lhsT,
                            rhs,
                            start=(kt == 0),
                            stop=(kt == KT - 1),
                        )
                for nt in range(NT):
                    ot = opool.tile([P, 512], f32, tag="ot")
                    nc.vector.tensor_copy(ot[:], accs[nt][:])
                    nc.sync.dma_start(
                        out_d[bpc * P:(bpc + 1) * P, nt * 512:(nt + 1) * 512],
                        ot[:],
                    )

    nc.compile()
    _CACHE["nc"] = nc
    return nc


def _prep_inputs(field_feat: np.ndarray, weights: np.ndarray):
    field_np = mybir.dt.np(mybir.dt(FIELD_DT))
    wd_np = mybir.dt.np(mybir.dt(WD_DT))
    field_feat = np.ascontiguousarray(field_feat, dtype=np.float32)
    weights = np.ascontiguousarray(weights, dtype=np.float32)

    # WD[io, nt, i, j32, s2] = weights[io*128+i, nt*32+j32, s2 % 16]
    wdd = np.concatenate([weights, weights], axis=-1)          # [256, 256, 32]
    wd = wdd.reshape(IO, P, NT, JPN, 2 * K).transpose(0, 2, 1, 3, 4)
    wd = np.ascontiguousarray(wd, dtype=wd_np)

    in_maps = []
    for c in range(N_CORES):
        # fieldT[kt, i128, bp]; kt = x*IO + io over (x, io), i128 minor of i
        fT = field_feat[c].transpose(2, 1, 0)                  # [16x, 256i, 512bp]
        fT = fT.reshape(K, IO, P, NUM_PART).reshape(KT, P, NUM_PART)
        in_maps.append({"fieldT": np.ascontiguousarray(fT, dtype=field_np), "wd": wd})
    return in_maps


def kernel(field_feat: np.ndarray, weights: np.ndarray) -> np.ndarray:
    nc = _build()
    in_maps = _prep_inputs(field_feat, weights)
    trace = bool(int(os.environ.get("KERNEL_TRACE", "0")))
    res = run_bass_kernel_spmd(nc, in_maps, list(range(N_CORES)), trace=trace)
    if trace:
        kernel.last_exec_time_ns = res.exec_time_ns
        kernel.last_results = res
    out = np.stack([res.results[c]["out"] for c in range(N_CORES)], axis=0)
    return out.reshape(BATCH, NUM_PART, OUT_FEAT, K)


# revision 6
# speedup vs baseline: 1.3551x; 1.1891x over previous
"""Trainium2 Bass kernel for nn_EquiLinearRegToReg.

Math: out[b,p,j,y] = sum_{i,x} weights[i, j, (y-x)%K] * field_feat[b,p,i,x]
Shapes: field_feat [8, 512, 256, 16] f32, weights [256, 256, 16] f32
        -> out [8, 512, 256, 16] f32.

Strategy: data-parallel over batch (1 batch of M=512 rows per core).
Per core this is a [512, 4096] @ [4096, 4096] matmul where the right
operand is the block-circulant expansion of weights. The 16 circular
shifts are materialized on the host (32 MB in fp16) and STREAMED from
DRAM as contiguous slabs, so every matmul's moving operand is a fully
contiguous [128, 2*32*16] AP (a strided windowed AP costs +25ns/MM in
AP-walk overhead; contiguous hits the 216ns/MM pair floor at N=512).

Inputs are cast to fp16 on the host: fp32r matmuls self-load weights
(+107ns/MM serialized); fp16 gets a separate FWL LDWEIGHTS that the PE
pulls ahead of in-flight matmuls, so weight loads are free. PSUM
accumulation is fp32; fp16 mantissa (10 bits) keeps the result within
~3e-4 relative error.

Loop structure: 4 groups of 8 PSUM banks (2 out-column tiles x 4
row-chunks); each group accumulates over all 32 K-tiles, then evicts
PSUM->SBUF->DRAM while the next group computes.
"""

import os
import numpy as np

import concourse.bass as bass
import concourse.mybir as mybir
import concourse.tile as tile
from concourse import bacc
from concourse.bass_utils import run_bass_kernel_spmd

BATCH, NUM_PART, IN_FEAT, OUT_FEAT, K = 8, 512, 256, 256, 16
N_CORES = 8
P = 128
IO = IN_FEAT // P          # 2 partition-tiles over in_features
KT = K * IO                # 32 K-tiles of 128, kt = io*16 + x
BPC = NUM_PART // P        # 4 chunks of 128 rows
NT = OUT_FEAT * K // 512   # 8 output column tiles of 512
JPN = OUT_FEAT // NT       # 32 j's per output tile
NG = 4                     # groups of 2 column tiles
NTL = NT // NG             # column tiles per group (2)

_CACHE = {}


def _build():
    """Build + compile the per-core Bass program (cached)."""
    if "nc" in _CACHE:
        return _CACHE["nc"]

    f32 = mybir.dt.float32
    f16 = mybir.dt.float16

    nc = bacc.Bacc(None, target_bir_lowering=False, debug=False)
    # fieldT[kt, i128, bp] : K-major transposed field shard, kt = io*16 + x
    field_d = nc.dram_tensor("fieldT", [KT, P, NUM_PART], f16, kind="ExternalInput")
    # wd[g, kt, i128, ntl, j32, y16] : pre-shifted weight slabs
    wd_d = nc.dram_tensor("wd", [NG, KT, P, NTL, JPN, K], f16, kind="ExternalInput")
    out_d = nc.dram_tensor("out", [NUM_PART, OUT_FEAT * K], f32, kind="ExternalOutput")

    with tile.TileContext(nc) as tc:
        with (
            tc.tile_pool(name="fpool", bufs=1) as fpool,
            tc.tile_pool(name="wpool", bufs=40) as wpool,
            tc.tile_pool(name="opool", bufs=8) as opool,
            tc.tile_pool(name="psum", bufs=8, space="PSUM") as psum,
        ):
            ft = fpool.tile([P, KT, NUM_PART], f16)

            # Group-0 weight slabs interleaved with field slabs so the
            # kt-sweep can start immediately and stays ahead of DMA.
            ws0 = []
            for kt in range(KT):
                w = wpool.tile([P, NTL, JPN, K], f16, tag="ws", name=f"ws0_{kt}")
                nc.sync.dma_start(w[:], wd_d[0, kt])
                ws0.append(w)
                nc.sync.dma_start(ft[:, kt, :], field_d[kt])

            for g in range(NG):
                accs = [
                    psum.tile([P, 512], f32, tag="ps", name=f"ps_{g}_{i}")
                    for i in range(BPC * NTL)
                ]
                for kt in range(KT):
                    if g == 0:
                        w = ws0[kt]
                    else:
                        w = wpool.tile([P, NTL, JPN, K], f16, tag="ws",
                                       name=f"ws{g}_{kt}")
                        nc.sync.dma_start(w[:], wd_d[g, kt])
                    for bpc in range(BPC):
                        lhsT = ft[:, kt, bpc * P:(bpc + 1) * P]
                        for l in range(NTL):
                            nc.tensor.matmul(
                                accs[bpc * NTL + l][:],
                                lhsT,
                                w[:, l],
                                start=(kt == 0),
                                stop=(kt == KT - 1),
                            )
                for bpc in range(BPC):
                    for l in range(NTL):
                        nt = g * NTL + l
                        ot = opool.tile([P, 512], f32, tag="ot",
                                        name=f"ot_{g}_{bpc}_{l}")
                        nc.vector.tensor_copy(ot[:], accs[bpc * NTL + l][:])
                        nc.sync.dma_start(
                            out_d[bpc * P:(bpc + 1) * P, nt * 512:(nt + 1) * 512],
                            ot[:],
                        )

    nc.compile()
    _CACHE["nc"] = nc
    return nc


def _prep_inputs(field_feat: np.ndarray, weights: np.ndarray):
    field_feat = np.ascontiguousarray(field_feat, dtype=np.float32)
    weights = np.ascontiguousarray(weights, dtype=np.float32)

    # rolled[x, i, j, y] = weights[i, j, (y-x) % K]
    rolled = np.stack([np.roll(weights, x, axis=2) for x in range(K)])
    # wd[g, io*K+x, i128, l, j, y] = rolled[x, io*128+i128, (g*NTL+l)*JPN+j, y]
    wd = rolled.reshape(K, IO, P, NG, NTL, JPN, K).transpose(3, 1, 0, 2, 4, 5, 6)
    wd = np.ascontiguousarray(wd.reshape(NG, KT, P, NTL, JPN, K), dtype=np.float16)

    in_maps = []
    for c in range(N_CORES):
        # fieldT[io*K+x, i128, bp]
        fT = field_feat[c].transpose(1, 2, 0)                  # [256i, 16x, 512bp]
        fT = fT.reshape(IO, P, K, NUM_PART).transpose(0, 2, 1, 3)
        fT = np.ascontiguousarray(fT.reshape(KT, P, NUM_PART), dtype=np.float16)
        in_maps.append({"fieldT": fT, "wd": wd})
    return in_maps


def kernel(field_feat: np.ndarray, weights: np.ndarray) -> np.ndarray:
    nc = _build()
    in_maps = _prep_inputs(field_feat, weights)
    trace = bool(int(os.environ.get("KERNEL_TRACE", "0")))
    res = run_bass_kernel_spmd(nc, in_maps, list(range(N_CORES)), trace=trace)
    if trace:
        kernel.last_exec_time_ns = res.exec_time_ns
        kernel.last_results = res
    out = np.stack([res.results[c]["out"] for c in range(N_CORES)], axis=0)
    return out.reshape(BATCH, NUM_PART, OUT_FEAT, K)
